# revision 9
# baseline (speedup 1.0000x reference)
"""Trainium2 Bass kernel for nn_Conv2d_72430328481302.

Conv2d: input (16,128,56,56) f32, weight (128,128,3,3), bias (128),
stride 1, pad 1, dilation 1 -> output (16,128,56,56).

Strategy (Winograd F(2,3) along H):
  - Data-parallel over batch: 2 images per core across 8 cores, weight
    replicated.  Host pre-pads each image to a [Cin=128, 58x58]
    zero-framed bf16 plane so input DMA is contiguous row chunks.
  - The 3x3 conv is decomposed as Winograd F(2,3) along H x direct
    along W: for each pair of output rows (2t, 2t+1), with padded
    input rows r = P[2t..2t+3]:
        V0 = r0 - r2, V1 = r1 + r2, V2 = r2 - r1, V3 = r1 - r3
    and the PE accumulates, per j in 0..3, M_j = sum_kw U[j,kw] @
    V_j[:, kw:kw+56] (f32 PSUM).  U[j,kw] = sum_kh G[j,kh] w[:,:,kh,kw]
    is host-precomputed.  Output rows are then
        y[2t]   = M0 + M1 + M2 + b
        y[2t+1] = M1 - M2 - M3 + b
    12 matmul-columns per 2 output rows instead of direct conv's 18 -
    a 1.5x cut in PE time (the baseline's PE ran saturated).
  - V0/V3 are never materialized per-slab: both are strided row views
    of ONE difference plane D[r] = P[r] - P[r+2] (V0[t] = D[2t],
    V3[t] = D[2t+1], every D row used exactly once), computed in a few
    large contiguous DVE ops - cheaper than 2 strided ops per slab.
  - Per 8-pair slab (448 psum cols): 12 bf16 matmuls into 4 PSUM banks
    (2 slabs in flight = 8 banks).  The output transform is spread so
    it hides under the PE: scalar ACTs s0=M0+b, m2=M2, s1=-M3+b;
    vector t0=M1+s0, t1=M1+s1 (plus the D/V ops for upcoming slabs);
    gpsimd (no PSUM port, SBUF-only) Y0=t0+m2, Y1=t1-m2 with strided
    bf16 row writes into the output plane.  The last slab's Y ops run
    on vector instead (shorter drain chain).
  - Images split into pair-chunks 4,8,8,8 / 8,8,8,4: small first slab
    starts the PE sooner during the DVFS ramp (warmup matmuls on raw
    scratch keep the PE busy from preamble-end, as in the baseline),
    small last slab shortens the drain.  Slab-0 transforms are split
    across vector and gpsimd so the first real matmul isn't serialized
    behind one engine.  Inputs ride the sync ring in 3 chunks/image;
    weights lead on the scalar ring; outputs leave per-slab on the
    sync ring (tail halves crossed sync/scalar).
"""

import os
import sys

for _p in ("/opt/trn_rl_repo",):
    if os.path.isdir(_p) and _p not in sys.path:
        sys.path.insert(0, _p)

import ml_dtypes
import numpy as np

import concourse.bass as bass
import concourse.tile as tile
from concourse import bacc, mybir
from concourse.bass_utils import run_bass_kernel_spmd

N_CORES = 8
N_IMGS = 16
IPC = N_IMGS // N_CORES  # images per core
CIN = 128
COUT = 128
H = W = 56
WP = 58  # padded width (1 col each side)
HP = 58  # padded height (1 row each side)
FLATP = HP * WP  # 3364
PAD_ALLOC = 3376  # pad to a 32 B multiple (bf16)
NPAIR = H // 2  # 28 output row-pairs per image
VROW = WP
VPLANE = NPAIR * VROW  # 1624 cols per V j-plane
# D plane: 56 rows of 58; odd-row strided matmul views reach
# (2*24+1)*58 + 8*116 = 3770 for the (24,4) slab -> pad alloc
D_ROWS = H
D_ALLOC = 3776  # >= (2*20+1)*58 + 8*116 = 3306, 32 B multiple
OUT_ALLOC = H * W + W  # odd-row strided view slack
F32 = mybir.dt.float32
BF16 = mybir.dt.bfloat16

# pair-chunks (p0, npairs) per image
SLABS = [
    (0, 0, 4), (0, 4, 8), (0, 12, 8), (0, 20, 8),
    (1, 0, 8), (1, 8, 8), (1, 16, 8), (1, 24, 4),
]
# input row chunks per image
CHUNKS = [(0, 26), (26, 42), (42, HP)]
# D-plane row chunks [r0, r1) with the input chunk each depends on
DCHUNKS = [(0, 8), (8, 24), (24, 40), (40, 56)]

_CACHE = {}


def _build_nc():
    nc = bacc.Bacc(
        "TRN2",
        target_bir_lowering=False,
        debug=False,
        num_devices=N_CORES,
    )
    x = nc.dram_tensor("x", [IPC, CIN, PAD_ALLOC], BF16, kind="ExternalInput")
    wt = nc.dram_tensor("wt", [CIN, 12, COUT], BF16, kind="ExternalInput")
    bvec = nc.dram_tensor("bvec", [COUT, 1], F32, kind="ExternalInput")
    y = nc.dram_tensor("y", [IPC, COUT, H * W], BF16, kind="ExternalOutput")

    # raw (non-pool) scratch for PE warmup (see baseline rationale)
    scrw = nc.alloc_sbuf_tensor("scrw", [CIN, 576], BF16)

    def row_view(ap_flat, off, nrows, pitch, width):
        return ap_flat[:, off : off + nrows * pitch].rearrange(
            "c (r k) -> c r k", k=pitch
        )[:, :, 0:width]

    with tile.TileContext(nc) as tc:
        with (
            tc.tile_pool(name="const", bufs=1) as cpool,
            tc.tile_pool(name="xin", bufs=1) as xpool,
            tc.tile_pool(name="vtr", bufs=1) as vpool,
            tc.tile_pool(name="yout", bufs=1) as ypool,
            tc.tile_pool(name="evac", bufs=10) as epool,
            tc.tile_pool(name="psum", bufs=8, space="PSUM") as pspool,
        ):
            wps = pspool.tile([COUT, 448], F32, name="wps", tag="ps")
            for wi in range(7):
                nc.tensor.matmul(
                    wps[:],
                    scrw.ap()[:, 0:128],
                    scrw.ap()[:, 128:576],
                    start=True, stop=True,
                )
            for wi in range(4):
                nc.tensor.matmul(
                    wps[:, 0:128],
                    scrw.ap()[:, 0:128],
                    scrw.ap()[:, 128:256],
                    start=True, stop=True,
                )

            wt_sb = cpool.tile([CIN, 12, COUT], BF16, name="wt_sb", tag="wt_sb")
            bias_sb = cpool.tile([COUT, 1], F32, name="bias_sb", tag="bias_sb")
            P, D, V, out_sb = {}, {}, {}, {}
            for i in range(IPC):
                P[i] = xpool.tile(
                    [CIN, PAD_ALLOC], BF16, name=f"P{i}", tag=f"P{i}"
                )
                D[i] = xpool.tile(
                    [CIN, D_ALLOC], BF16, name=f"D{i}", tag=f"D{i}"
                )
                V[i] = vpool.tile(
                    [CIN, 2, VPLANE], BF16, name=f"V{i}", tag=f"V{i}"
                )
                out_sb[i] = ypool.tile(
                    [COUT, OUT_ALLOC], BF16, name=f"out{i}", tag=f"out{i}"
                )

            # weights: the j=0 piece gates the first matmuls and leads;
            # the rest follows as one piece (lands before matmul 4 needs
            # it - the scalar ring runs parallel to the sync ring)
            nc.scalar.dma_start(wt_sb[:, 0:3, :], wt.ap()[:, 0:3, :])
            nc.scalar.dma_start(bias_sb[:], bvec.ap()[:])
            nc.scalar.dma_start(wt_sb[:, 3:12, :], wt.ap()[:, 3:12, :])

            # input chunks on the sync ring, image-major
            for i in range(IPC):
                for r0, r1 in CHUNKS:
                    e0 = r0 * WP
                    e1 = r1 * WP if r1 < HP else PAD_ALLOC
                    nc.sync.dma_start(P[i][:, e0:e1], x.ap()[i, :, e0:e1])

            def d_op(i, c):
                # D[r] = P[r] - P[r+2] over row chunk c: flat contiguous
                r0, r1 = DCHUNKS[c]
                a, b = r0 * WP, r1 * WP
                nc.vector.tensor_sub(
                    D[i][:, a:b], P[i][:, a:b],
                    P[i][:, a + 2 * WP : b + 2 * WP],
                )

            def v_op(engine, i, p0, npr, j):
                # V1 = r1 + r2 (j==1) / V2 = r2 - r1 (j==2) for a slab
                dst = row_view(V[i][:, j - 1, :], p0 * VROW, npr, VROW, VROW)
                r1v = row_view(P[i], (2 * p0 + 1) * WP, npr, 2 * WP, WP)
                r2v = row_view(P[i], (2 * p0 + 2) * WP, npr, 2 * WP, WP)
                if j == 1:
                    engine.tensor_add(dst, r1v, r2v)
                else:
                    engine.tensor_sub(dst, r2v, r1v)

            # prime slab 0/1 transforms before any t-ops enter the
            # vector queue; slab-0's three ops split vector/gpsimd so
            # the first real matmul isn't serialized behind one engine
            i0, p0_, n0 = SLABS[0]
            d_op(i0, 0)                      # vector: D[0,8) img0
            v_op(nc.gpsimd, i0, p0_, n0, 1)  # gpsimd: V1 slab0
            v_op(nc.vector, i0, p0_, n0, 2)  # vector: V2 slab0
            d_op(i0, 1)                      # vector: D[8,24) img0
            i1, p1_, n1 = SLABS[1]
            v_op(nc.vector, i1, p1_, n1, 1)
            v_op(nc.vector, i1, p1_, n1, 2)

            # per-slab prep work (for slab k, emitted during slab k-2):
            # remaining D chunks and V ops, all on vector
            def prep(k):
                i, p0, npr = SLABS[k]
                if k == 2:
                    d_op(0, 2)
                elif k == 3:
                    d_op(0, 3)
                elif k == 4:
                    d_op(1, 0)
                    d_op(1, 1)
                elif k == 5:
                    d_op(1, 2)
                elif k == 6:
                    d_op(1, 3)
                v_op(nc.vector, i, p0, npr, 1)
                v_op(nc.vector, i, p0, npr, 2)

            pend_dma = []
            for k, (i, p0, npr) in enumerate(SLABS):
                ncols = npr * W
                last = k == len(SLABS) - 1
                ps = [
                    pspool.tile([COUT, ncols], F32, name=f"ps{k}_{j}", tag="ps")
                    for j in range(4)
                ]
                for j in range(4):
                    if j == 0:
                        vv = row_view(D[i], 2 * p0 * WP, npr, 2 * WP, WP)
                    elif j == 3:
                        vv = row_view(D[i], (2 * p0 + 1) * WP, npr, 2 * WP, WP)
                    else:
                        vv = row_view(V[i][:, j - 1, :], p0 * VROW, npr,
                                      VROW, VROW)
                    for kw in range(3):
                        nc.tensor.matmul(
                            ps[j][:],
                            wt_sb[:, 3 * j + kw, :],
                            vv[:, :, kw : kw + W],
                            start=(kw == 0),
                            stop=(kw == 2),
                        )

                s0 = epool.tile([COUT, ncols], BF16, name=f"s0_{k}", tag="ev")
                s1 = epool.tile([COUT, ncols], BF16, name=f"s1_{k}", tag="ev")
                m2 = epool.tile([COUT, ncols], BF16, name=f"m2_{k}", tag="ev")
                t0 = epool.tile([COUT, ncols], BF16, name=f"t0_{k}", tag="ev")
                t1 = epool.tile([COUT, ncols], BF16, name=f"t1_{k}", tag="ev")
                # scalar in M-readiness order: M0 (3rd mm), M2 (9th),
                # M3 (12th)
                nc.scalar.activation(
                    s0[:], ps[0][:],
                    mybir.ActivationFunctionType.Identity,
                    bias=bias_sb[:, :],
                )
                nc.scalar.activation(
                    m2[:], ps[2][:],
                    mybir.ActivationFunctionType.Identity,
                )
                nc.scalar.activation(
                    s1[:], ps[3][:],
                    mybir.ActivationFunctionType.Identity,
                    bias=bias_sb[:, :], scale=-1.0,
                )
                nc.vector.tensor_add(t0[:], ps[1][:], s0[:])
                nc.vector.tensor_add(t1[:], ps[1][:], s1[:])
                if k + 2 < len(SLABS):
                    prep(k + 2)
                c0 = 2 * p0 * W
                y0 = row_view(out_sb[i], c0, npr, 2 * W, W)
                y1 = row_view(out_sb[i], c0 + W, npr, 2 * W, W)
                m2v = m2[:].rearrange("c (r k) -> c r k", k=W)
                t0v = t0[:].rearrange("c (r k) -> c r k", k=W)
                t1v = t1[:].rearrange("c (r k) -> c r k", k=W)
                # last slab: vector (faster op, shorter drain chain);
                # otherwise gpsimd so vector stays ahead on t/V work
                yeng = nc.vector if last else nc.gpsimd
                yeng.tensor_add(y0, t0v, m2v)
                yeng.tensor_sub(y1, t1v, m2v)

                pend_dma.append((y.ap()[i, :, c0 : c0 + ncols * 2],
                                 out_sb[i][:, c0 : c0 + ncols * 2]))
                if len(pend_dma) > 1:
                    dst, src = pend_dma.pop(0)
                    nc.sync.dma_start(dst, src)

            # final slab: halves crossed over sync/scalar rings
            dst, src = pend_dma.pop(0)
            half = 2 * SLABS[-1][2] * W // 2
            nc.sync.dma_start(dst[:, 0:half], src[:, 0:half],
                              single_packet=True)
            nc.scalar.dma_start(dst[:, half:], src[:, half:],
                                single_packet=True)

    nc.compile()
    return nc


def _get_nc():
    if "nc" not in _CACHE:
        _CACHE["nc"] = _build_nc()
    return _CACHE["nc"]


def _make_in_maps(input, weight, bias):
    input = np.asarray(input)
    weight = np.asarray(weight)
    bias = np.asarray(bias)
    padded = np.zeros((N_IMGS, CIN, PAD_ALLOC), dtype=ml_dtypes.bfloat16)
    pv = padded[:, :, :FLATP].reshape(N_IMGS, CIN, HP, WP)
    pv[:, :, 1 : H + 1, 1 : W + 1] = input
    # weight (Cout,Cin,3,3) -> Winograd F(2,3) transform along kh, then
    # lhsT layout (Cin, j*3+kw, Cout)
    G = np.array(
        [[1, 0, 0], [0.5, 0.5, 0.5], [0.5, -0.5, 0.5], [0, 0, 1]],
        dtype=np.float32,
    )
    U = np.einsum("jh,oihw->ijwo", G,
                  weight.astype(np.float32))  # [Cin, 4, 3, Cout]
    wt_host = np.ascontiguousarray(
        U.reshape(CIN, 12, COUT)
    ).astype(ml_dtypes.bfloat16)
    b_host = np.ascontiguousarray(bias.reshape(COUT, 1), dtype=np.float32)
    return [
        {
            "x": padded[c * IPC : (c + 1) * IPC],
            "wt": wt_host,
            "bvec": b_host,
        }
        for c in range(N_CORES)
    ]


def run(input, weight, bias, trace=False, tmpdir=None):
    """Run the SPMD kernel; returns (output, BassKernelResults)."""
    nc = _get_nc()
    in_maps = _make_in_maps(input, weight, bias)
    res = run_bass_kernel_spmd(
        nc, in_maps, list(range(N_CORES)), trace=trace, tmpdir=tmpdir
    )
    out = np.concatenate(
        [np.asarray(res.results[c]["y"]) for c in range(N_CORES)], axis=0
    ).astype(np.float32)
    return out.reshape(N_IMGS, COUT, H, W).astype(np.float32), res


def kernel(input, weight, bias):
    out, _ = run(input, weight, bias, trace=False)
    return out


# revision 10
# speedup vs baseline: 1.0808x; 1.0808x over previous
"""Trainium2 Bass kernel for nn_Conv2d_72430328481302.

Conv2d: input (16,128,56,56) f32, weight (128,128,3,3), bias (128),
stride 1, pad 1, dilation 1 -> output (16,128,56,56).

Strategy (Winograd F(2,3) along H):
  - Data-parallel over batch: 2 images per core across 8 cores, weight
    replicated.  Host pre-pads each image to a [Cin=128, 58x58]
    zero-framed bf16 plane so input DMA is contiguous row chunks.
  - The 3x3 conv is decomposed as Winograd F(2,3) along H x direct
    along W: for each pair of output rows (2t, 2t+1), with padded
    input rows r = P[2t..2t+3]:
        V0 = r0 - r2, V1 = r1 + r2, V2 = r2 - r1, V3 = r1 - r3
    and the PE accumulates, per j in 0..3, M_j = sum_kw U[j,kw] @
    V_j[:, kw:kw+56] (f32 PSUM).  U[j,kw] = sum_kh G[j,kh] w[:,:,kh,kw]
    is host-precomputed.  Output rows are then
        y[2t]   = M0 + M1 + M2 + b
        y[2t+1] = M1 - M2 - M3 + b
    12 matmul-columns per 2 output rows instead of direct conv's 18 -
    a 1.5x cut in PE time (the baseline's PE ran saturated).
  - V0/V3 are never materialized per-slab: both are strided row views
    of ONE difference plane D[r] = P[r] - P[r+2] (V0[t] = D[2t],
    V3[t] = D[2t+1], every D row used exactly once), computed in a few
    large contiguous DVE ops - cheaper than 2 strided ops per slab.
  - Per 8-pair slab (448 psum cols): 12 bf16 matmuls into 4 PSUM banks
    (2 slabs in flight = 8 banks).  The output transform is spread so
    it hides under the PE: scalar ACTs s0=M0+b, m2=M2, s1=-M3+b;
    vector t0=M1+s0, t1=M1+s1 (plus the D/V ops for upcoming slabs);
    gpsimd (no PSUM port, SBUF-only) Y0=t0+m2, Y1=t1-m2 with strided
    bf16 row writes into the output plane.  The last slab's Y ops run
    on vector instead (shorter drain chain).
  - Images split into pair-chunks 4,8,8,8 / 8,8,8,4: small first slab
    starts the PE sooner during the DVFS ramp (warmup matmuls on raw
    scratch keep the PE busy from preamble-end, as in the baseline),
    small last slab shortens the drain.  Slab-0 transforms are split
    across vector and gpsimd so the first real matmul isn't serialized
    behind one engine.  Inputs ride the sync ring in 3 chunks/image;
    weights lead on the scalar ring; outputs leave per-slab on the
    sync ring (tail halves crossed sync/scalar).
"""

import os
import sys

for _p in ("/opt/trn_rl_repo",):
    if os.path.isdir(_p) and _p not in sys.path:
        sys.path.insert(0, _p)

import ml_dtypes
import numpy as np

import concourse.bass as bass
import concourse.tile as tile
from concourse import bacc, mybir
from concourse.bass_utils import run_bass_kernel_spmd

N_CORES = 8
N_IMGS = 16
IPC = N_IMGS // N_CORES  # images per core
CIN = 128
COUT = 128
H = W = 56
WP = 58  # padded width (1 col each side)
HP = 58  # padded height (1 row each side)
FLATP = HP * WP  # 3364
PAD_ALLOC = 3376  # pad to a 32 B multiple (bf16)
NPAIR = H // 2  # 28 output row-pairs per image
VROW = WP
VPLANE = NPAIR * VROW  # 1624 cols per V j-plane
# D plane: 56 rows of 58; odd-row strided matmul views reach
# (2*24+1)*58 + 8*116 = 3770 for the (24,4) slab -> pad alloc
D_ROWS = H
D_ALLOC = 3776  # >= (2*20+1)*58 + 8*116 = 3306, 32 B multiple
OUT_ALLOC = H * W + W  # odd-row strided view slack
F32 = mybir.dt.float32
BF16 = mybir.dt.bfloat16

# pair-chunks (p0, npairs) per image
SLABS = [
    (0, 0, 4), (0, 4, 8), (0, 12, 8), (0, 20, 8),
    (1, 0, 8), (1, 8, 8), (1, 16, 8), (1, 24, 4),
]
# input row chunks per image: tiny first chunk (the early DMA clock is
# slow, and chunk 0 gates the first real matmul), finer later chunks so
# no slab waits on one big late transfer
CHUNKS = [(0, 10), (10, 18), (18, 26), (26, 34), (34, 42), (42, 50),
          (50, HP)]
# D-plane row chunks [r0, r1) with the input chunk each depends on
DCHUNKS = [(0, 8), (8, 24), (24, 40), (40, 56)]

_CACHE = {}


def _build_nc():
    nc = bacc.Bacc(
        "TRN2",
        target_bir_lowering=False,
        debug=False,
        num_devices=N_CORES,
    )
    x = nc.dram_tensor("x", [IPC, CIN, PAD_ALLOC], BF16, kind="ExternalInput")
    wt = nc.dram_tensor("wt", [CIN, 12, COUT], BF16, kind="ExternalInput")
    bvec = nc.dram_tensor("bvec", [COUT, 1], F32, kind="ExternalInput")
    y = nc.dram_tensor("y", [IPC, COUT, H * W], BF16, kind="ExternalOutput")

    # raw (non-pool) scratch for PE warmup (see baseline rationale)
    scrw = nc.alloc_sbuf_tensor("scrw", [CIN, 576], BF16)

    def row_view(ap_flat, off, nrows, pitch, width):
        return ap_flat[:, off : off + nrows * pitch].rearrange(
            "c (r k) -> c r k", k=pitch
        )[:, :, 0:width]

    with tile.TileContext(nc) as tc:
        with (
            tc.tile_pool(name="const", bufs=1) as cpool,
            tc.tile_pool(name="xin", bufs=1) as xpool,
            tc.tile_pool(name="vtr", bufs=1) as vpool,
            tc.tile_pool(name="yout", bufs=1) as ypool,
            tc.tile_pool(name="evac", bufs=10) as epool,
            tc.tile_pool(name="psum", bufs=8, space="PSUM") as pspool,
        ):
            wps = pspool.tile([COUT, 448], F32, name="wps", tag="ps")
            for wi in range(7):
                nc.tensor.matmul(
                    wps[:],
                    scrw.ap()[:, 0:128],
                    scrw.ap()[:, 128:576],
                    start=True, stop=True,
                )
            for wi in range(4):
                nc.tensor.matmul(
                    wps[:, 0:128],
                    scrw.ap()[:, 0:128],
                    scrw.ap()[:, 128:256],
                    start=True, stop=True,
                )

            wt_sb = cpool.tile([CIN, 12, COUT], BF16, name="wt_sb", tag="wt_sb")
            bias_sb = cpool.tile([COUT, 1], F32, name="bias_sb", tag="bias_sb")
            P, D, V, out_sb = {}, {}, {}, {}
            for i in range(IPC):
                P[i] = xpool.tile(
                    [CIN, PAD_ALLOC], BF16, name=f"P{i}", tag=f"P{i}"
                )
                D[i] = xpool.tile(
                    [CIN, D_ALLOC], BF16, name=f"D{i}", tag=f"D{i}"
                )
                V[i] = vpool.tile(
                    [CIN, 2, VPLANE], BF16, name=f"V{i}", tag=f"V{i}"
                )
                out_sb[i] = ypool.tile(
                    [COUT, OUT_ALLOC], BF16, name=f"out{i}", tag=f"out{i}"
                )

            # weights: the j=0 piece gates the first matmuls and leads;
            # the rest follows as one piece (lands before matmul 4 needs
            # it - the scalar ring runs parallel to the sync ring)
            nc.scalar.dma_start(wt_sb[:, 0:3, :], wt.ap()[:, 0:3, :])
            nc.scalar.dma_start(bias_sb[:], bvec.ap()[:])
            nc.scalar.dma_start(wt_sb[:, 3:12, :], wt.ap()[:, 3:12, :])

            # input chunks on the sync ring, image-major
            for i in range(IPC):
                for r0, r1 in CHUNKS:
                    e0 = r0 * WP
                    e1 = r1 * WP if r1 < HP else PAD_ALLOC
                    nc.sync.dma_start(P[i][:, e0:e1], x.ap()[i, :, e0:e1])

            def d_op(i, c):
                # D[r] = P[r] - P[r+2] over row chunk c: flat contiguous
                r0, r1 = DCHUNKS[c]
                a, b = r0 * WP, r1 * WP
                nc.vector.tensor_sub(
                    D[i][:, a:b], P[i][:, a:b],
                    P[i][:, a + 2 * WP : b + 2 * WP],
                )

            def v_op(engine, i, p0, npr, j):
                # V1 = r1 + r2 (j==1) / V2 = r2 - r1 (j==2) for a slab
                dst = row_view(V[i][:, j - 1, :], p0 * VROW, npr, VROW, VROW)
                r1v = row_view(P[i], (2 * p0 + 1) * WP, npr, 2 * WP, WP)
                r2v = row_view(P[i], (2 * p0 + 2) * WP, npr, 2 * WP, WP)
                if j == 1:
                    engine.tensor_add(dst, r1v, r2v)
                else:
                    engine.tensor_sub(dst, r2v, r1v)

            # prime slab 0/1 transforms before any t-ops enter the
            # vector queue; slab-0's three ops split vector/gpsimd so
            # the first real matmul isn't serialized behind one engine
            i0, p0_, n0 = SLABS[0]
            d_op(i0, 0)                      # vector: D[0,8) img0
            v_op(nc.gpsimd, i0, p0_, n0, 1)  # gpsimd: V1 slab0
            v_op(nc.vector, i0, p0_, n0, 2)  # vector: V2 slab0
            d_op(i0, 1)                      # vector: D[8,24) img0
            i1, p1_, n1 = SLABS[1]
            v_op(nc.vector, i1, p1_, n1, 1)
            v_op(nc.vector, i1, p1_, n1, 2)

            # per-slab prep work (for slab k, emitted during slab k-2):
            # remaining D chunks and V ops, all on vector
            def prep(k):
                i, p0, npr = SLABS[k]
                if k == 2:
                    d_op(0, 2)
                elif k == 3:
                    d_op(0, 3)
                elif k == 4:
                    d_op(1, 0)
                    d_op(1, 1)
                elif k == 5:
                    d_op(1, 2)
                elif k == 6:
                    d_op(1, 3)
                v_op(nc.vector, i, p0, npr, 1)
                v_op(nc.vector, i, p0, npr, 2)

            pend_dma = []
            for k, (i, p0, npr) in enumerate(SLABS):
                ncols = npr * W
                last = k == len(SLABS) - 1
                ps = [
                    pspool.tile([COUT, ncols], F32, name=f"ps{k}_{j}", tag="ps")
                    for j in range(4)
                ]
                for j in range(4):
                    if j == 0:
                        vv = row_view(D[i], 2 * p0 * WP, npr, 2 * WP, WP)
                    elif j == 3:
                        vv = row_view(D[i], (2 * p0 + 1) * WP, npr, 2 * WP, WP)
                    else:
                        vv = row_view(V[i][:, j - 1, :], p0 * VROW, npr,
                                      VROW, VROW)
                    for kw in range(3):
                        nc.tensor.matmul(
                            ps[j][:],
                            wt_sb[:, 3 * j + kw, :],
                            vv[:, :, kw : kw + W],
                            start=(kw == 0),
                            stop=(kw == 2),
                        )

                s0 = epool.tile([COUT, ncols], BF16, name=f"s0_{k}", tag="ev")
                s1 = epool.tile([COUT, ncols], BF16, name=f"s1_{k}", tag="ev")
                m2 = epool.tile([COUT, ncols], BF16, name=f"m2_{k}", tag="ev")
                t0 = epool.tile([COUT, ncols], BF16, name=f"t0_{k}", tag="ev")
                t1 = epool.tile([COUT, ncols], BF16, name=f"t1_{k}", tag="ev")
                # scalar in M-readiness order: M0 (3rd mm), M2 (9th),
                # M3 (12th)
                nc.scalar.activation(
                    s0[:], ps[0][:],
                    mybir.ActivationFunctionType.Identity,
                    bias=bias_sb[:, :],
                )
                nc.scalar.activation(
                    m2[:], ps[2][:],
                    mybir.ActivationFunctionType.Identity,
                )
                nc.scalar.activation(
                    s1[:], ps[3][:],
                    mybir.ActivationFunctionType.Identity,
                    bias=bias_sb[:, :], scale=-1.0,
                )
                nc.vector.tensor_add(t0[:], ps[1][:], s0[:])
                nc.vector.tensor_add(t1[:], ps[1][:], s1[:])
                if k + 2 < len(SLABS):
                    prep(k + 2)
                c0 = 2 * p0 * W
                y0 = row_view(out_sb[i], c0, npr, 2 * W, W)
                y1 = row_view(out_sb[i], c0 + W, npr, 2 * W, W)
                m2v = m2[:].rearrange("c (r k) -> c r k", k=W)
                t0v = t0[:].rearrange("c (r k) -> c r k", k=W)
                t1v = t1[:].rearrange("c (r k) -> c r k", k=W)
                # last slab: vector (faster op, shorter drain chain);
                # otherwise gpsimd so vector stays ahead on t/V work
                yeng = nc.vector if last else nc.gpsimd
                yeng.tensor_add(y0, t0v, m2v)
                yeng.tensor_sub(y1, t1v, m2v)

                pend_dma.append((y.ap()[i, :, c0 : c0 + ncols * 2],
                                 out_sb[i][:, c0 : c0 + ncols * 2]))
                if len(pend_dma) > 1:
                    dst, src = pend_dma.pop(0)
                    nc.sync.dma_start(dst, src)

            # final slab: halves crossed over sync/scalar rings
            dst, src = pend_dma.pop(0)
            half = 2 * SLABS[-1][2] * W // 2
            nc.sync.dma_start(dst[:, 0:half], src[:, 0:half],
                              single_packet=True)
            nc.scalar.dma_start(dst[:, half:], src[:, half:],
                                single_packet=True)

    nc.compile()
    return nc


def _get_nc():
    if "nc" not in _CACHE:
        _CACHE["nc"] = _build_nc()
    return _CACHE["nc"]


def _make_in_maps(input, weight, bias):
    input = np.asarray(input)
    weight = np.asarray(weight)
    bias = np.asarray(bias)
    padded = np.zeros((N_IMGS, CIN, PAD_ALLOC), dtype=ml_dtypes.bfloat16)
    pv = padded[:, :, :FLATP].reshape(N_IMGS, CIN, HP, WP)
    pv[:, :, 1 : H + 1, 1 : W + 1] = input
    # weight (Cout,Cin,3,3) -> Winograd F(2,3) transform along kh, then
    # lhsT layout (Cin, j*3+kw, Cout)
    G = np.array(
        [[1, 0, 0], [0.5, 0.5, 0.5], [0.5, -0.5, 0.5], [0, 0, 1]],
        dtype=np.float32,
    )
    U = np.einsum("jh,oihw->ijwo", G,
                  weight.astype(np.float32))  # [Cin, 4, 3, Cout]
    wt_host = np.ascontiguousarray(
        U.reshape(CIN, 12, COUT)
    ).astype(ml_dtypes.bfloat16)
    b_host = np.ascontiguousarray(bias.reshape(COUT, 1), dtype=np.float32)
    return [
        {
            "x": padded[c * IPC : (c + 1) * IPC],
            "wt": wt_host,
            "bvec": b_host,
        }
        for c in range(N_CORES)
    ]


def run(input, weight, bias, trace=False, tmpdir=None):
    """Run the SPMD kernel; returns (output, BassKernelResults)."""
    nc = _get_nc()
    in_maps = _make_in_maps(input, weight, bias)
    res = run_bass_kernel_spmd(
        nc, in_maps, list(range(N_CORES)), trace=trace, tmpdir=tmpdir
    )
    out = np.concatenate(
        [np.asarray(res.results[c]["y"]) for c in range(N_CORES)], axis=0
    ).astype(np.float32)
    return out.reshape(N_IMGS, COUT, H, W).astype(np.float32), res


def kernel(input, weight, bias):
    out, _ = run(input, weight, bias, trace=False)
    return out
